# revision 23
# baseline (speedup 1.0000x reference)
"""Trainium2 Bass kernel for nn_CoordsToNRF.

out[b, p] = atom_nc[b, p] * (AU2KCALMOLA / MAX_NRF) / ||coords[b, I[p]] - coords[b, J[p]]||^2

Design (pure data parallel over batch, 8 cores x 128 batches):
  - Pair gather+subtract on the TensorEngine per xyz component:
        D_c = CT_c.T @ S        (S = +-2^-7 tril pair-selection matrix)
    Coords use a TWO-term f16 split accumulated in f32 PSUM: the shared
    smat plane holds +-2^-7 and the per-term scales are folded into the
    f16 coords stationaries (ct0*2^7 exact, ct1*2^-7).  6 matmuls per
    512-pair group; measured back-to-back N=512 f16 matmuls pace at
    ~216ns when the PE isn't dependency-stalled.
  - PE prewarm: dummy matmuls issued before any DMA lands so the HAM
    clock gate reaches 2.4 GHz before real work.
  - ScalarE: one activation per 512-group squares all 3 planes
    ([128,3,512] PSUM read across 3 banks), scale folds 2/K; bf16 out
    written into a 2048-wide macro tile.
  - VectorE at 2048-wide macro granularity (amortizes the ~280cyc/instr
    overhead): add1 bf16+bf16->bf16, add2 bf16+bf16->f32 (mixed f32
    INPUT is the slow path - both-bf16-in is fast for any output),
    reciprocal_approx_fast f32.
  - GpSimd: final anc*inv multiply (anc pre-doubled on host) at 2048 wide.
  - IO: atom_nc f16, output bf16 (f16 would overflow) upcast on host.
    Simulated max rel err on the seed-0 data: 1.07e-2 vs the 2e-2 gate.
"""

import sys

for _p in ("/opt/trn_rl_repo",):
    if _p not in sys.path:
        sys.path.insert(0, _p)

import numpy as np
import ml_dtypes
from contextlib import ExitStack

import concourse.bass as bass
import concourse.bacc as bacc
import concourse.tile as tile
from concourse import mybir
from concourse.bass_utils import run_bass_kernel_spmd

F32 = mybir.dt.float32
F16 = mybir.dt.float16
BF16 = mybir.dt.bfloat16

N_ATOMS = 128
NC2 = N_ATOMS * (N_ATOMS - 1) // 2  # 8128
BATCH = 1024
N_CORES = 8
BPC = BATCH // N_CORES  # 128

AU2KCALMOLA = 627.5095 * 0.529177
MAX_NRF = 13036.0
K_CONST = AU2KCALMOLA / MAX_NRF
SQ_SCALE = float(np.sqrt(2.0 / K_CONST))  # fold 2/K into the squares
LO_SHIFT = 2.0 ** 14
S_SCALE = 2.0 ** -7

GROUP = 512
MACRO = 1024  # elementwise/store granularity (2 groups)
MACROS = [(m, min(MACRO, NC2 - m)) for m in range(0, NC2, MACRO)]
CHUNK = 2048  # input-load granularity
CHUNKS = [(c, min(CHUNK, NC2 - c)) for c in range(0, NC2, CHUNK)]
# smat arrives in a small early piece then big chunks
S_CHUNKS = [(0, 512), (512, 1536)] + [(c, min(CHUNK, NC2 - c)) for c in range(2048, NC2, CHUNK)]

_I, _J = np.tril_indices(N_ATOMS, -1)



def _build_smat16() -> np.ndarray:
    s = np.zeros((N_ATOMS, NC2), dtype=np.float32)
    p = np.arange(NC2)
    s[_I, p] = S_SCALE
    s[_J, p] = -S_SCALE
    return s.astype(np.float16)


def _split_coords2(coords32: np.ndarray):
    c64 = coords32.astype(np.float64)
    c0 = c64.astype(np.float16)
    r1 = (c64 - c0.astype(np.float64)) * LO_SHIFT
    c1 = r1.astype(np.float16)
    ct0p = (c0.astype(np.float32) * 2.0 ** 7).astype(np.float16)
    ct1p = (c1.astype(np.float32) * 2.0 ** -7).astype(np.float16)
    return ct0p, ct1p


def _build_program():
    nc = bacc.Bacc("TRN2", target_bir_lowering=False, debug=False)

    ct_d = [
        nc.dram_tensor(f"ct{t}", [N_ATOMS, 3 * BPC], F16, kind="ExternalInput")
        for t in range(2)
    ]
    smat_d = nc.dram_tensor("smat", [N_ATOMS, NC2], F16, kind="ExternalInput")
    ident_d = nc.dram_tensor("ident", [128, 128], BF16, kind="ExternalInput")
    anc_d = nc.dram_tensor("atom_nc", [BPC, NC2], F16, kind="ExternalInput")
    out_d = nc.dram_tensor("out", [BPC, NC2], BF16, kind="ExternalOutput")

    with tile.TileContext(nc) as tc, ExitStack() as ctx:
        const = ctx.enter_context(tc.tile_pool(name="const", bufs=1))
        sqp = ctx.enter_context(tc.tile_pool(name="sqp", bufs=3))
        sqp2 = ctx.enter_context(tc.tile_pool(name="sqp2", bufs=3))
        ewp = ctx.enter_context(tc.tile_pool(name="ewp", bufs=6))
        outp = ctx.enter_context(tc.tile_pool(name="outp", bufs=3))
        ps_d = ctx.enter_context(tc.tile_pool(name="ps_d", bufs=2, space="PSUM"))
        ps_r = ctx.enter_context(tc.tile_pool(name="ps_r", bufs=2, space="PSUM"))

        # ---- inputs ----
        ident_sb = const.tile([128, 128], BF16, tag="ident")
        nc.sync.dma_start(ident_sb[:], ident_d[:, :])
        ct_sb = []
        for t in range(2):
            cs = const.tile([N_ATOMS, 3, BPC], F16, tag=f"ct{t}")
            nc.sync.dma_start(cs[:], ct_d[t][:, :].rearrange("a (c b) -> a c b", c=3))
            ct_sb.append(cs)

        smat_parts = []
        for ci, (c0, cw) in enumerate(S_CHUNKS):
            st = const.tile([N_ATOMS, cw], F16, tag=f"smat{ci}")
            nc.sync.dma_start(st[:], smat_d[:, c0:c0 + cw])
            smat_parts.append((c0, cw, st))
        anc_sb = []
        for ci, (c0, cw) in enumerate(CHUNKS):
            at = const.tile([BPC, cw], F16, tag=f"anc{ci}")
            nc.sync.dma_start(at[:], anc_d[:, c0:c0 + cw])
            anc_sb.append(at)

        def smat_slice(g0, fd):
            for c0, cw, st in smat_parts:
                if c0 <= g0 and g0 + fd <= c0 + cw:
                    return st[:, g0 - c0:g0 - c0 + fd]
            raise AssertionError("no smat part")

        # ---- main loop: group pairs; DVE ops batched by type so that
        # independent same-type ops dual-issue in the DVE pipeline ----
        GROUPS = [(g, min(GROUP, NC2 - g)) for g in range(0, NC2, GROUP)]
        for pi in range(0, len(GROUPS), 2):
            pair = GROUPS[pi:pi + 2]
            sqs, t01s, r2s, invs = [], [], [], []
            for gj, (gs, fd) in enumerate(pair):
                d3 = ps_d.tile([128, 3, GROUP], F32, tag="d3")
                rhs = smat_slice(gs, fd)
                for t in range(2):
                    for c in range(3):
                        nc.tensor.matmul(
                            d3[:, c, :fd], ct_sb[t][:, c, :], rhs,
                            start=(t == 0), stop=(t == 1),
                            skip_group_check=True,
                        )
                sq = (sqp if gj == 0 else sqp2).tile(
                    [128, 3, GROUP], BF16, tag="sq")
                nc.scalar.activation(
                    sq[:, :, :fd], d3[:, :, :fd],
                    mybir.ActivationFunctionType.Square,
                    bias=0.0, scale=SQ_SCALE,
                )
                sqs.append(sq)
            for gj, (gs, fd) in enumerate(pair):
                t01 = ewp.tile([128, GROUP], BF16, tag="t01")
                nc.vector.tensor_add(t01[:, :fd], sqs[gj][:, 0, :fd], sqs[gj][:, 1, :fd])
                # r2 = t01 + sq_z summed exactly on the PE (identity matmuls
                # accumulating in f32 PSUM) - the DVE's bf16->f32 add runs at
                # the slow 1x errata path, the PE does it in 2x215ns
                r2 = ps_r.tile([128, GROUP], F32, tag="r2")
                nc.tensor.matmul(r2[:, :fd], ident_sb[:], t01[:, :fd],
                                 start=True, stop=False, skip_group_check=True)
                nc.tensor.matmul(r2[:, :fd], ident_sb[:], sqs[gj][:, 2, :fd],
                                 start=False, stop=True, skip_group_check=True)
                inv = ewp.tile([128, GROUP], F32, tag="inv")
                nc.vector.reciprocal_approx_fast(inv[:, :fd], r2[:, :fd])
                invs.append(inv)
            o_tile = outp.tile([128, MACRO], BF16)
            for gj, (gs, fd) in enumerate(pair):
                ci, coff = gs // CHUNK, gs % CHUNK
                nc.gpsimd.tensor_mul(
                    o_tile[:, gj * GROUP:gj * GROUP + fd],
                    anc_sb[ci][:, coff:coff + fd], invs[gj][:, :fd])
            ms = pair[0][0]
            mw = sum(fd for _, fd in pair)
            nc.sync.dma_start(out_d[:, ms:ms + mw], o_tile[:, :mw])

    nc.compile()
    return nc


_CACHED = None


def _get_program():
    global _CACHED
    if _CACHED is None:
        _CACHED = _build_program()
    return _CACHED


def kernel(coords, atom_nc, _trace=False, _trace_kwargs=None):
    coords = np.ascontiguousarray(np.asarray(coords, dtype=np.float32))
    atom_nc = np.ascontiguousarray(np.asarray(atom_nc, dtype=np.float32))
    assert coords.shape == (BATCH, N_ATOMS, 3)
    assert atom_nc.shape == (BATCH, NC2)

    nc = _get_program()
    smat = _build_smat16()
    c0, c1 = _split_coords2(coords)
    anc16 = (atom_nc * 2.0).astype(np.float16)
    ident = np.eye(128, dtype=ml_dtypes.bfloat16)

    in_maps = []
    for core in range(N_CORES):
        b0 = core * BPC
        ct0 = np.ascontiguousarray(
            c0[b0:b0 + BPC].transpose(1, 2, 0).reshape(N_ATOMS, 3 * BPC))
        ct1 = np.ascontiguousarray(
            c1[b0:b0 + BPC].transpose(1, 2, 0).reshape(N_ATOMS, 3 * BPC))
        in_maps.append({
            "ct0": ct0,
            "ct1": ct1,
            "smat": smat,
            "ident": ident,
            "atom_nc": anc16[b0:b0 + BPC],
        })

    kw = {}
    if _trace:
        kw["trace"] = True
        kw.update(_trace_kwargs or {})
    res = run_bass_kernel_spmd(nc, in_maps, core_ids=list(range(N_CORES)), **kw)
    out = np.concatenate(
        [np.asarray(r["out"]).astype(np.float32) for r in res.results], axis=0)
    if _trace:
        return out, res
    return out


if __name__ == "__main__":
    rng = np.random.default_rng(0)
    coords = (rng.standard_normal((BATCH, N_ATOMS, 3)) * 5.0).astype(np.float32)
    atom_nc = rng.uniform(1.0, 50.0, (BATCH, NC2)).astype(np.float32)
    out = kernel(coords, atom_nc)
    print(out.shape, out.dtype)


# revision 24
# speedup vs baseline: 1.0706x; 1.0706x over previous
"""Trainium2 Bass kernel for nn_CoordsToNRF.

out[b, p] = atom_nc[b, p] * (AU2KCALMOLA / MAX_NRF) / ||coords[b, I[p]] - coords[b, J[p]]||^2

Design (pure data parallel over batch, 8 cores x 128 batches):
  - Pair gather+subtract on the TensorEngine per xyz component:
        D_c = CT_c.T @ S        (S = +-2^-7 tril pair-selection matrix)
    Coords use a TWO-term f16 split accumulated in f32 PSUM: the shared
    smat plane holds +-2^-7 and the per-term scales are folded into the
    f16 coords stationaries (ct0*2^7 exact, ct1*2^-7).  6 matmuls per
    512-pair group; measured back-to-back N=512 f16 matmuls pace at
    ~216ns when the PE isn't dependency-stalled.
  - PE prewarm: dummy matmuls issued before any DMA lands so the HAM
    clock gate reaches 2.4 GHz before real work.
  - ScalarE: one activation per 512-group squares all 3 planes
    ([128,3,512] PSUM read across 3 banks), scale folds 2/K; bf16 out
    written into a 2048-wide macro tile.
  - VectorE at 2048-wide macro granularity (amortizes the ~280cyc/instr
    overhead): add1 bf16+bf16->bf16, add2 bf16+bf16->f32 (mixed f32
    INPUT is the slow path - both-bf16-in is fast for any output),
    reciprocal_approx_fast f32.
  - GpSimd: final anc*inv multiply (anc pre-doubled on host) at 2048 wide.
  - IO: atom_nc f16, output bf16 (f16 would overflow) upcast on host.
    Simulated max rel err on the seed-0 data: 1.07e-2 vs the 2e-2 gate.
"""

import sys

for _p in ("/opt/trn_rl_repo",):
    if _p not in sys.path:
        sys.path.insert(0, _p)

import numpy as np
import ml_dtypes
from contextlib import ExitStack

import concourse.bass as bass
import concourse.bacc as bacc
import concourse.tile as tile
from concourse import mybir
from concourse.bass_utils import run_bass_kernel_spmd

F32 = mybir.dt.float32
F16 = mybir.dt.float16
BF16 = mybir.dt.bfloat16

N_ATOMS = 128
NC2 = N_ATOMS * (N_ATOMS - 1) // 2  # 8128
BATCH = 1024
N_CORES = 8
BPC = BATCH // N_CORES  # 128

AU2KCALMOLA = 627.5095 * 0.529177
MAX_NRF = 13036.0
K_CONST = AU2KCALMOLA / MAX_NRF
SQ_SCALE = float(np.sqrt(2.0 / K_CONST))  # fold 2/K into the squares
LO_SHIFT = 2.0 ** 14
S_SCALE = 2.0 ** -7

GROUP = 512
MACRO = 1024  # elementwise/store granularity (2 groups)
MACROS = [(m, min(MACRO, NC2 - m)) for m in range(0, NC2, MACRO)]
CHUNK = 2048  # input-load granularity
CHUNKS = [(c, min(CHUNK, NC2 - c)) for c in range(0, NC2, CHUNK)]
# smat arrives in a small early piece then big chunks
S_CHUNKS = [(0, 512), (512, 1536)] + [(c, min(CHUNK, NC2 - c)) for c in range(2048, NC2, CHUNK)]

_I, _J = np.tril_indices(N_ATOMS, -1)



def _build_smat16() -> np.ndarray:
    s = np.zeros((N_ATOMS, NC2), dtype=np.float32)
    p = np.arange(NC2)
    s[_I, p] = S_SCALE
    s[_J, p] = -S_SCALE
    return s.astype(np.float16)


def _split_coords2(coords32: np.ndarray):
    c64 = coords32.astype(np.float64)
    c0 = c64.astype(np.float16)
    r1 = (c64 - c0.astype(np.float64)) * LO_SHIFT
    c1 = r1.astype(np.float16)
    ct0p = (c0.astype(np.float32) * 2.0 ** 7).astype(np.float16)
    ct1p = (c1.astype(np.float32) * 2.0 ** -7).astype(np.float16)
    return ct0p, ct1p


def _build_program():
    nc = bacc.Bacc("TRN2", target_bir_lowering=False, debug=False)

    ct_d = [
        nc.dram_tensor(f"ct{t}", [N_ATOMS, 3 * BPC], F16, kind="ExternalInput")
        for t in range(2)
    ]
    smat_d = nc.dram_tensor("smat", [N_ATOMS, NC2], F16, kind="ExternalInput")
    anc_d = nc.dram_tensor("atom_nc", [BPC, NC2], F16, kind="ExternalInput")
    out_d = nc.dram_tensor("out", [BPC, NC2], BF16, kind="ExternalOutput")

    with tile.TileContext(nc) as tc, ExitStack() as ctx:
        const = ctx.enter_context(tc.tile_pool(name="const", bufs=1))
        sqp = ctx.enter_context(tc.tile_pool(name="sqp", bufs=3))
        sqp2 = ctx.enter_context(tc.tile_pool(name="sqp2", bufs=3))
        ewp = ctx.enter_context(tc.tile_pool(name="ewp", bufs=6))
        outp = ctx.enter_context(tc.tile_pool(name="outp", bufs=3))
        ps_w = ctx.enter_context(tc.tile_pool(name="ps_w", bufs=1, space="PSUM"))
        ps_d = ctx.enter_context(tc.tile_pool(name="ps_d", bufs=2, space="PSUM"))

        # ---- inputs ----
        # ---- PE prewarm (HAM un-throttle during the input DMA window) ----
        junk = const.tile([128, GROUP], F16, tag="junk")
        nc.vector.memset(junk[:], 0)
        pw = ps_w.tile([128, GROUP], F32)
        for _ in range(6):
            nc.tensor.matmul(
                pw[:], junk[:, :128], junk[:], start=True, stop=True,
                skip_group_check=True,
            )

        ct_sb = []
        for t in range(2):
            cs = const.tile([N_ATOMS, 3, BPC], F16, tag=f"ct{t}")
            nc.sync.dma_start(cs[:], ct_d[t][:, :].rearrange("a (c b) -> a c b", c=3))
            ct_sb.append(cs)

        smat_parts = []
        for ci, (c0, cw) in enumerate(S_CHUNKS):
            st = const.tile([N_ATOMS, cw], F16, tag=f"smat{ci}")
            nc.sync.dma_start(st[:], smat_d[:, c0:c0 + cw])
            smat_parts.append((c0, cw, st))
        anc_sb = []
        for ci, (c0, cw) in enumerate(CHUNKS):
            at = const.tile([BPC, cw], F16, tag=f"anc{ci}")
            nc.sync.dma_start(at[:], anc_d[:, c0:c0 + cw])
            anc_sb.append(at)

        def smat_slice(g0, fd):
            for c0, cw, st in smat_parts:
                if c0 <= g0 and g0 + fd <= c0 + cw:
                    return st[:, g0 - c0:g0 - c0 + fd]
            raise AssertionError("no smat part")

        # ---- main loop: group pairs; DVE ops batched by type so that
        # independent same-type ops dual-issue in the DVE pipeline ----
        GROUPS = [(g, min(GROUP, NC2 - g)) for g in range(0, NC2, GROUP)]
        for pi in range(0, len(GROUPS), 2):
            pair = GROUPS[pi:pi + 2]
            sqs, t01s, r2s, invs = [], [], [], []
            for gj, (gs, fd) in enumerate(pair):
                d3 = ps_d.tile([128, 3, GROUP], F32, tag="d3")
                rhs = smat_slice(gs, fd)
                for t in range(2):
                    for c in range(3):
                        nc.tensor.matmul(
                            d3[:, c, :fd], ct_sb[t][:, c, :], rhs,
                            start=(t == 0), stop=(t == 1),
                            skip_group_check=True,
                        )
                sq = (sqp if gj == 0 else sqp2).tile(
                    [128, 3, GROUP], BF16, tag="sq")
                nc.scalar.activation(
                    sq[:, :, :fd], d3[:, :, :fd],
                    mybir.ActivationFunctionType.Square,
                    bias=0.0, scale=SQ_SCALE,
                )
                sqs.append(sq)
            for gj, (gs, fd) in enumerate(pair):
                t01 = ewp.tile([128, GROUP], BF16, tag="t01")
                nc.vector.tensor_add(t01[:, :fd], sqs[gj][:, 0, :fd], sqs[gj][:, 1, :fd])
                r2 = ewp.tile([128, GROUP], F32, tag="r2")
                nc.vector.tensor_add(r2[:, :fd], t01[:, :fd], sqs[gj][:, 2, :fd])
                inv = ewp.tile([128, GROUP], F32, tag="inv")
                nc.vector.reciprocal_approx_fast(inv[:, :fd], r2[:, :fd])
                invs.append(inv)
            o_tile = outp.tile([128, MACRO], BF16)
            for gj, (gs, fd) in enumerate(pair):
                ci, coff = gs // CHUNK, gs % CHUNK
                nc.gpsimd.tensor_mul(
                    o_tile[:, gj * GROUP:gj * GROUP + fd],
                    anc_sb[ci][:, coff:coff + fd], invs[gj][:, :fd])
            ms = pair[0][0]
            mw = sum(fd for _, fd in pair)
            nc.sync.dma_start(out_d[:, ms:ms + mw], o_tile[:, :mw])

    nc.compile()
    return nc


_CACHED = None


def _get_program():
    global _CACHED
    if _CACHED is None:
        _CACHED = _build_program()
    return _CACHED


def kernel(coords, atom_nc, _trace=False, _trace_kwargs=None):
    coords = np.ascontiguousarray(np.asarray(coords, dtype=np.float32))
    atom_nc = np.ascontiguousarray(np.asarray(atom_nc, dtype=np.float32))
    assert coords.shape == (BATCH, N_ATOMS, 3)
    assert atom_nc.shape == (BATCH, NC2)

    nc = _get_program()
    smat = _build_smat16()
    c0, c1 = _split_coords2(coords)
    anc16 = (atom_nc * 2.0).astype(np.float16)

    in_maps = []
    for core in range(N_CORES):
        b0 = core * BPC
        ct0 = np.ascontiguousarray(
            c0[b0:b0 + BPC].transpose(1, 2, 0).reshape(N_ATOMS, 3 * BPC))
        ct1 = np.ascontiguousarray(
            c1[b0:b0 + BPC].transpose(1, 2, 0).reshape(N_ATOMS, 3 * BPC))
        in_maps.append({
            "ct0": ct0,
            "ct1": ct1,
            "smat": smat,
            "atom_nc": anc16[b0:b0 + BPC],
        })

    kw = {}
    if _trace:
        kw["trace"] = True
        kw.update(_trace_kwargs or {})
    res = run_bass_kernel_spmd(nc, in_maps, core_ids=list(range(N_CORES)), **kw)
    out = np.concatenate(
        [np.asarray(r["out"]).astype(np.float32) for r in res.results], axis=0)
    if _trace:
        return out, res
    return out


if __name__ == "__main__":
    rng = np.random.default_rng(0)
    coords = (rng.standard_normal((BATCH, N_ATOMS, 3)) * 5.0).astype(np.float32)
    atom_nc = rng.uniform(1.0, 50.0, (BATCH, NC2)).astype(np.float32)
    out = kernel(coords, atom_nc)
    print(out.shape, out.dtype)
